# revision 39
# baseline (speedup 1.0000x reference)
"""Trainium2 Bass kernel: causal GQA self-attention, RoPE + QK RMS-norm, bf16.

Sharding over 8 NeuronCores: core = 4*b + g (b in {0,1} batch, g in {0..3}
kv-group). Each core computes its 4 q heads + 1 kv head and the partial
c_proj output y_heads @ wproj[:, 512g:512g+512].T of shape [T, C]; the host
sums the 4 partials per batch (the "all-reduce after c_proj" at gather time).

Schedule (deep software pipeline): stage A of strip 4j+4+h is emitted inside
stage B(j) between heads h and h+1, so the serial per-strip chain
(PE proj -> DVE rope+rsqrt -> xbar transpose) hides under the PE-heavy
attention sweeps and every engine queue keeps flowing. Choices that matter:
 - rsqrt for the QK RMS-norm runs ENTIRELY on DVE (bit-trick seed + 2
   Newton steps on [128,5] tiles): the ACT engine then only ever runs
   Exp/Copy, which share one activation-table set -> zero ACT_TABLE_LOADs
   after the first.
 - causal mask is an additive -30000 matmul folded into the scores psum
   accumulation group on PE (no post-exp masking work on DVE/ACT).
 - q transposes are batched: one 3D-dst xbar DMA per strip covers all 4
   heads ([t,(h d)] -> [d,h,t]).
 - output stores are 4 batched [128,2048] DMAs per slice on the gpsimd
   (SWDGE) queue; cos/sin loads ride the same queue ahead of them; the SP
   (HWDGE) queue carries only x strips + transposes. In-order DMA queues
   head-of-line block on data dependencies, so queue assignment is part of
   the schedule.
 - fp8 (DoubleRow) paths exist but are OFF: every naive e4m3 insertion
   measures ~2.8e-2 rel err alone (gate 2e-2); error-feedback splits cost
   back the 2x.
"""

import math
from contextlib import ExitStack

import numpy as np

import concourse.bass as bass
import concourse.mybir as mybir
import concourse.tile as tile
from concourse import bacc
from concourse.bass import ts
from concourse.bass_utils import run_bass_kernel_spmd

F32 = mybir.dt.float32
I32 = mybir.dt.int32
BF16 = mybir.dt.bfloat16
FP8 = mybir.dt.float8e4
N_HEAD = 16
N_KV = 4
D = 128
RMS_EPS = float(np.finfo(np.float32).eps)
SCALE = 1.0 / math.sqrt(D)

FP8P = False         # x/wq/wkv in fp8, DoubleRow projections (see docstring)
FP8C = False         # wproj/ynj in fp8, DoubleRow c_proj
W_SCALE = 64.0
OUT_DT = "bf16"      # halves output-store DMA; measured rel err impact
                     # 3.6e-3 -> 4.5e-3 in the numpy quantization model
PEMASK = True        # mask = additive -30000 folded into scores matmul group

ALU = mybir.AluOpType
AF = mybir.ActivationFunctionType


def build_bass(T=2048, C=2048, HQ=4, E=2048, rep=1,
               pscfg=None, pbufs=16, obufs=3,
               stages='ABC', wsplit=4, gdma=True,
               defnorm=True, fp8p=None, fp8c=None, outdt=None,
               nrsqrt=True, deep=True, bigot=True, bxp=True,
               pemask=True, csq='gpsimd', stq='gpsimd', nriter=2,
               bfro=True):
    TT, CT, NE, TQ = T // 128, C // 128, E // 512, T // 512
    HD = HQ * 128
    if fp8p is None:
        fp8p = FP8P
    if fp8c is None:
        fp8c = FP8C
    if outdt is None:
        outdt = OUT_DT
    dt = BF16
    xdt = FP8 if fp8p else BF16
    ydt = FP8 if fp8c else BF16
    odt = F32 if outdt == "f32" else BF16
    os_val = (1.0 / W_SCALE) if fp8p else 1.0
    ot_scale = (1.0 / W_SCALE) if fp8c else None
    if pscfg is None:
        pscfg = (3, 1, 1, 2, 1)  # ps_s, pq, pkv, ps_a, ps_d (8 banks)
    sb_, qb_, kb_, ab_, db_ = pscfg

    nc = bacc.Bacc("TRN2", target_bir_lowering=False)
    xT_d = nc.dram_tensor("xT", [C, T], xdt, kind="ExternalInput")
    wqT_d = nc.dram_tensor("wqT", [C, HD], xdt, kind="ExternalInput")
    wkvT_d = nc.dram_tensor("wkvT", [C, 256], xdt, kind="ExternalInput")
    wpT_d = nc.dram_tensor("wpT", [HD, E], ydt, kind="ExternalInput")
    cos_d = nc.dram_tensor("cosd", [T, D], F32, kind="ExternalInput")
    sin_d = nc.dram_tensor("sind", [T, D], F32, kind="ExternalInput")
    mask_d = nc.dram_tensor("maskd", [128, 128], dt, kind="ExternalInput")
    id_d = nc.dram_tensor("identd", [128, 128], dt, kind="ExternalInput")
    out_d = nc.dram_tensor("out", [T, E], odt, kind="ExternalOutput")

    DR = mybir.MatmulPerfMode.DoubleRow
    assert pemask == PEMASK, "host mask content must match pemask"

    with tile.TileContext(nc) as tc, ExitStack() as ctx:
        P = lambda **kw: ctx.enter_context(tc.tile_pool(**kw))
        wp = P(name="w", bufs=1)            # persistent weights/constants
        xp = P(name="x", bufs=8)            # xT strips
        csp = P(name="cs", bufs=8)          # cos/sin tiles
        rp = P(name="rope", bufs=3)         # rope scratch
        qnp = P(name="qn", bufs=3)          # normalized q/k (pre-transpose)
        pp = P(name="p", bufs=pbufs)        # exp(P) tiles (all blocks alive)
        bp = P(name="bc", bufs=2)           # denominators / bcast
        yp = P(name="y", bufs=3)            # per-head unnormalized y^T
        op = P(name="o", bufs=obufs)        # output staging
        ps_s = P(name="ps_s", bufs=sb_, space="PSUM")   # scores/pb/cproj
        ps_q = P(name="ps_q", bufs=qb_, space="PSUM")   # q projection
        ps_k = P(name="ps_k", bufs=kb_, space="PSUM")   # kv projection
        ps_a = P(name="ps_a", bufs=ab_, space="PSUM")   # AV accumulators
        ps_d = P(name="ps_d", bufs=db_, space="PSUM")   # denominators

        engq = {"gpsimd": nc.gpsimd, "sp": nc.sync,
                "act": nc.scalar, "dve": nc.vector}
        cse = engq[csq] if gdma else nc.sync
        ste = engq[stq] if gdma else nc.sync

        xT_r = xT_d.ap().rearrange("(n p) t -> p n t", p=128)
        xs_t, cs_t = {}, {}

        csdt = dt if bfro else F32  # casting DMA needs the gpsimd queue
        assert not bfro or (gdma and csq == 'gpsimd')

        def load_strip(i):
            xs = xp.tile([128, CT, 128], xdt, name="xs", tag="xs")
            nc.sync.dma_start(xs, xT_r[:, :, ts(i, 128)])
            cst = csp.tile([128, D], csdt, tag="cos", name="cst")
            cse.dma_start(cst, cos_d.ap()[ts(i, 128), :])
            snt = csp.tile([128, D], csdt, tag="sin", name="snt")
            cse.dma_start(snt, sin_d.ap()[ts(i, 128), :])
            xs_t[i] = xs
            cs_t[i] = (cst, snt)

        # ---- startup DMAs: first strip + first weight chunk, then the
        # rest; strips 4..7 prefetch right behind so group 1's stage A
        # (interleaved into bc(0)) never waits on the SP queue ----
        load_strip(0)
        wq_s = wp.tile([128, CT, HD], xdt)
        wkv_s = wp.tile([128, CT, 256], xdt)
        wq_r = wqT_d.ap().rearrange("(n p) m -> p n m", p=128)
        wkv_r = wkvT_d.ap().rearrange("(n p) m -> p n m", p=128)
        csz = CT // wsplit
        nc.sync.dma_start(wq_s[:, 0:csz], wq_r[:, 0:csz])
        nc.sync.dma_start(wkv_s[:, 0:csz], wkv_r[:, 0:csz])
        mask_s = wp.tile([128, 128], dt)
        nc.sync.dma_start(mask_s, mask_d.ap())
        ident = wp.tile([128, 128], dt)
        nc.sync.dma_start(ident, id_d.ap())
        for w in range(1, wsplit):
            cs0 = w * csz
            nc.sync.dma_start(wq_s[:, cs0:cs0 + csz], wq_r[:, cs0:cs0 + csz])
            nc.sync.dma_start(wkv_s[:, cs0:cs0 + csz], wkv_r[:, cs0:cs0 + csz])
        for i in range(1, min(8, TT)):
            load_strip(i)
        wp_s = wp.tile([128, HQ, E], ydt)
        wp_r = wpT_d.ap().rearrange("(n p) m -> p n m", p=128)
        wp_loaded = [False]
        ones_c = wp.tile([128, 1], dt, name="ones_c", tag="ones_c")
        nc.vector.memset(ones_c, 1.0)
        ones_sq = wp.tile([128, 128], dt, name="ones_sq", tag="ones_sq")
        nc.vector.memset(ones_sq, os_val)
        eps_s = wp.tile([128, 1], F32)
        nc.vector.memset(eps_s, RMS_EPS)

        def bcast(ap, axis, n):
            a = list(ap.ap)
            a.insert(axis, [0, n])
            return bass.AP(tensor=ap.tensor, offset=ap.offset, ap=a)

        if True:  # tiles/closures shared by all reps (flat group stream)
            if bxp:
                qT = {j: wp.tile([128, HQ, 4, 128], dt, tag=f"qTj{j}",
                                 name=f"qTj{j}") for j in range(TQ)}
            else:
                qT = {}
                for h in range(HQ):
                    for j in range(TQ):
                        qT[(h, j)] = wp.tile([128, 4, 128], dt,
                                             tag=f"qT{h}_{j}",
                                             name=f"qT{h}_{j}")
            kT = [wp.tile([128, 128], dt, tag=f"kT{i}", name=f"kT{i}")
                  for i in range(TT)]
            vS = [wp.tile([128, 128], dt, tag=f"vS{i}", name=f"vS{i}")
                  for i in range(TT)]

            pend = []  # deferred transposes: (src_ap, dst)

            def drain_transposes():
                for src_ap, dst in pend:
                    nc.sync.dma_start(dst, src_ap, transpose=True)
                del pend[:]

            rdt = dt if bfro else F32

            def rope(src, nh, cst, snt, qn, qo):
                """src: [128, nh, 128] (bf16 sbuf if bfro, else psum f32);
                returns the rotated tile ro."""
                ro = rp.tile([128, nh, D], rdt, tag=f"ro{qo}", name="ro")
                nc.vector.tensor_mul(ro, src, bcast(cst[:, :], 1, nh))
                tmp = rp.tile([128, nh, 64], rdt, tag=f"tm{qo}", name="tmp")
                nc.vector.tensor_mul(tmp, src[:, :, 64:128],
                                     bcast(snt[:, 0:64], 1, nh))
                nc.vector.tensor_sub(ro[:, :, 0:64], ro[:, :, 0:64], tmp)
                tmp2 = rp.tile([128, nh, 64], rdt, tag=f"t2{qo}", name="tmp2")
                nc.vector.tensor_mul(tmp2, src[:, :, 0:64],
                                     bcast(snt[:, 64:128], 1, nh))
                nc.vector.tensor_add(ro[:, :, 64:128], ro[:, :, 64:128],
                                     tmp2)
                return ro

            MAGIC = 0x5F3759DF

            def nr_rsqrt(rr, v):
                """rr = 1/sqrt(v) elementwise on DVE only ([128, n] tiles).

                Bit-trick seed y0 via (M2 + ~i) >> 1 (= magic - (i>>1) up to
                1 ulp of the seed), then `nriter` Newton steps; no ACT
                involvement so the activation table stays on the Exp set.
                """
                n = v.shape[1]
                y = rp.tile([128, n], F32, tag="nr_y", name="nr_y")
                vb = v.bitcast(I32)
                yb = y.bitcast(I32)
                # walrus requires op0/op1 of one tensor_scalar to share a
                # class (bitwise vs arith), and there is no reversed
                # subtract: use magic - (i>>1) = ~(i>>1) + (magic+1)
                nc.vector.tensor_scalar(yb, vb, 1, 0xFFFFFFFF,
                                        ALU.logical_shift_right,
                                        ALU.bitwise_xor)
                nc.vector.tensor_scalar(yb, yb, MAGIC + 1, None, ALU.add)
                t = rp.tile([128, n], F32, tag="nr_t", name="nr_t")
                for _ in range(nriter):
                    nc.vector.tensor_mul(t, y, y)
                    nc.vector.tensor_mul(t, t, v)
                    nc.vector.tensor_scalar(t, t, -0.5, 1.5,
                                            ALU.mult, ALU.add)
                    nc.vector.tensor_mul(y, y, t)
                nc.vector.tensor_copy(rr, y)

            def stage_a(i):
                drain_transposes()
                xs = xs_t[i]
                cst, snt = cs_t[i]
                if not wp_loaded[0] and i >= 1:
                    # all HQ wproj chunks must be queued on the SP FIFO
                    # before bc(0)'s output stores (stage C(0) reads them)
                    hi = HQ if i >= 3 else i
                    for w in range(i - 1, hi):
                        nc.sync.dma_start(wp_s[:, w], wp_r[:, w])
                    if i >= 3:
                        wp_loaded[0] = True

                pq = ps_q.tile([128, HD], F32, tag="pq", name="pq")
                pkv = ps_k.tile([128, 256], F32, tag="pkv", name="pkv")
                if fp8p:
                    n2 = CT // 2
                    for c in range(n2):
                        sl = slice(2 * c, 2 * c + 2)
                        nc.tensor.matmul(pq, xs[:, sl], wq_s[:, sl],
                                         start=(c == 0), stop=(c == n2 - 1),
                                         perf_mode=DR)
                        nc.tensor.matmul(pkv, xs[:, sl], wkv_s[:, sl],
                                         start=(c == 0), stop=(c == n2 - 1),
                                         perf_mode=DR)
                else:
                    for c in range(CT):
                        nc.tensor.matmul(pq, xs[:, c], wq_s[:, c],
                                         start=(c == 0), stop=(c == CT - 1))
                        nc.tensor.matmul(pkv, xs[:, c], wkv_s[:, c],
                                         start=(c == 0), stop=(c == CT - 1))
                nc.scalar.copy(vS[i], pkv[:, 128:256])
                j, tsub = i // 4, i % 4

                if bfro:
                    # ACT copies psum->sbuf bf16 up front: frees the psum
                    # bank after ~0.6us and every rope op then runs in the
                    # DVE 2-byte fast mode on SBUF
                    qsrc = rp.tile([128, HQ, D], dt, tag="qsrc", name="qsrc")
                    nc.scalar.copy(
                        qsrc.rearrange("p a b -> p (a b)"), pq[:])
                    ksrc = rp.tile([128, 1, D], dt, tag="ksrc", name="ksrc")
                    nc.scalar.copy(
                        ksrc.rearrange("p a b -> p (a b)"), pkv[:, 0:128])
                    q_in = qsrc[:]
                    k_in = ksrc[:]
                else:
                    q_in = pq[:].rearrange("p (h d) -> p h d", d=D)
                    k_in = pkv[:, 0:128].rearrange("p (h d) -> p h d", d=D)

                ro_q = rope(q_in, HQ, cst, snt, None, 0)
                ro_k = rope(k_in, 1, cst, snt, None, 8)
                sq5 = rp.tile([128, HQ + 1], rdt, tag="sq5", name="sq5")
                scr = rp.tile([128, HQ, D], rdt, tag="scr", name="scr")
                nc.vector.tensor_mul(scr, ro_q, ro_q)
                scrk = rp.tile([128, 1, D], rdt, tag="scrk", name="scrk")
                nc.vector.tensor_mul(scrk, ro_k, ro_k)
                with nc.allow_low_precision(
                        "rms sumsq in bf16: 0.4% on the q/k scale, "
                        "measured 4.5e-3 -> 5.4e-3 end to end"):
                    nc.vector.reduce_sum(sq5[:, 0:HQ], scr,
                                         axis=mybir.AxisListType.X)
                    nc.vector.reduce_sum(sq5[:, HQ:HQ + 1], scrk,
                                         axis=mybir.AxisListType.X)
                rr5 = rp.tile([128, HQ + 1], F32, tag="rr5", name="rr5")
                if nrsqrt:
                    sq5f = rp.tile([128, HQ + 1], F32, tag="sq5f",
                                   name="sq5f")
                    nc.vector.tensor_scalar(sq5f, sq5, 1.0 / D, RMS_EPS,
                                            ALU.mult, ALU.add)
                    nr_rsqrt(rr5, sq5f)
                else:
                    nc.scalar.activation(rr5, sq5, AF.Abs_reciprocal_sqrt,
                                         bias=eps_s[:, :], scale=1.0 / D)

                qn = qnp.tile([128, HQ + 1, D], dt, tag="qn", name="qn")
                for h in range(HQ):
                    nc.vector.tensor_scalar_mul(qn[:, h], ro_q[:, h],
                                                rr5[:, h:h + 1])
                nc.vector.tensor_scalar_mul(qn[:, HQ], ro_k[:, 0],
                                            rr5[:, HQ:HQ + 1])
                if bxp:
                    pend.append((qn[:, 0:HQ].rearrange("p a b -> p (a b)"),
                                 qT[j][:, :, tsub]))
                    pend.append((qn[:, HQ], kT[i]))
                else:
                    for h in range(HQ):
                        pend.append((qn[:, h], qT[(h, j)][:, tsub]))
                    pend.append((qn[:, HQ], kT[i]))

            # ---- stage B + C per tq-slice ----
            def stage_bc(j, nxt, nxtl):
                nblk = 4 * j + 4
                ynj = yp.tile([128, HQ, 4, 128], ydt, tag="ynj", name="ynj")
                pend_norm = []  # deferred one head for slack

                def drain_norm():
                    for rdr_p, yv_p, h_p in pend_norm:
                        pb = ps_s.tile([128, 512], F32, tag="s", name="pb")
                        nc.tensor.matmul(pb, ones_sq[0:1, :], rdr_p)
                        nc.vector.tensor_mul(
                            ynj[:, h_p].rearrange("p a b -> p (a b)"),
                            yv_p, pb)
                    del pend_norm[:]

                for h in range(HQ):
                    if h < len(nxtl):
                        load_strip(nxtl[h])
                    pes = []
                    for i in range(nblk):
                        ai = max(0, i - 4 * j) * 128
                        diag = i >= 4 * j
                        psb = ps_s.tile([128, 512], F32, tag="s")
                        qmv = (qT[j][:, h, ai // 128:4] if bxp
                               else qT[(h, j)][:, ai // 128:4])
                        if pemask and diag:
                            nc.tensor.matmul(psb[:, ai:512], kT[i], qmv,
                                             start=True, stop=False)
                            nc.tensor.matmul(psb[:, ai:ai + 128], ident,
                                             mask_s, start=False, stop=True)
                        else:
                            nc.tensor.matmul(psb[:, ai:512], kT[i], qmv)
                        pe = pp.tile([128, 512], dt, tag="pe")
                        nc.scalar.activation(pe[:, ai:512], psb[:, ai:512],
                                             AF.Exp, scale=SCALE)
                        if diag and not pemask:
                            nc.vector.tensor_mul(pe[:, ai:ai + 128],
                                                 pe[:, ai:ai + 128], mask_s)
                        pes.append((pe, ai))
                    pdh = ps_d.tile([1, 512], F32)
                    for i, (pe, ai) in enumerate(pes):
                        nc.tensor.matmul(pdh[:, ai:512], ones_c,
                                         pe[:, ai:512],
                                         start=(i == 0), stop=(i == nblk - 1))
                    rd = bp.tile([1, 512], F32, tag="rd")
                    nc.vector.reciprocal(rd, pdh)
                    rdr = bp.tile([1, 512], dt, tag="rdr")
                    nc.vector.tensor_copy(rdr, rd)
                    if defnorm:
                        drain_norm()
                    pav = ps_a.tile([128, 512], F32)
                    for i, (pe, ai) in enumerate(pes):
                        nc.tensor.matmul(pav[:, ai:512], vS[i],
                                         pe[:, ai:512],
                                         start=(i == 0), stop=(i == nblk - 1))
                    yv = yp.tile([128, 512], dt, tag="yv", name="yv")
                    if h % 2 == 0:
                        nc.scalar.copy(yv, pav)
                    else:
                        nc.vector.tensor_copy(yv, pav)
                    pend_norm.append((rdr, yv, h))
                    if not defnorm or h == HQ - 1:
                        # last head: normalize BEFORE the interleaved
                        # stage_a so stage C's ynj read isn't queued on DVE
                        # behind a whole strip's rope chain
                        drain_norm()
                    if h < len(nxt):
                        stage_a(nxt[h])
                drain_norm()
                drain_transposes()
                if stages == 'AB':
                    dbg2 = op.tile([128, 512], F32, tag="dbg")
                    nc.vector.tensor_copy(
                        dbg2, ynj[:, 0].rearrange("p a b -> p (a b)"))
                    nc.sync.dma_start(out_d.ap()[ts(j, 128), 0:512], dbg2)
                    return
                for tsub in range(4):
                    otb = None
                    if bigot:
                        otb = op.tile([128, NE, 512], odt, tag="ot",
                                      name="otb")
                    for e in range(NE):
                        pc = ps_s.tile([128, 512], F32, tag="s", name="pc")
                        if fp8c:
                            for hp in range(HQ // 2):
                                nc.tensor.matmul(
                                    pc, ynj[:, 2 * hp:2 * hp + 2, tsub],
                                    wp_s[:, 2 * hp:2 * hp + 2, ts(e, 512)],
                                    start=(hp == 0), stop=(hp == HQ // 2 - 1),
                                    perf_mode=DR)
                        else:
                            for h in range(HQ):
                                nc.tensor.matmul(pc, ynj[:, h, tsub],
                                                 wp_s[:, h, ts(e, 512)],
                                                 start=(h == 0),
                                                 stop=(h == HQ - 1))
                        ot = otb[:, e] if bigot else op.tile(
                            [128, 512], odt, tag="ot", name="ot")
                        # all output copies on ACT: during stage C the DVE
                        # is the engine racing through next-group ropes
                        if ot_scale is None:
                            nc.scalar.copy(ot, pc)
                        else:
                            nc.scalar.mul(ot, pc, ot_scale)
                        if not bigot:
                            deng = (nc.sync if (e % 2 == 0 or not gdma)
                                    else nc.gpsimd)
                            deng.dma_start(
                                out_d.ap()[512 * j + 128 * tsub:
                                           512 * j + 128 * tsub + 128,
                                           ts(e, 512)], ot)
                    if bigot:
                        ste.dma_start(
                            out_d.ap()[512 * j + 128 * tsub:
                                       512 * j + 128 * tsub + 128, :]
                            .rearrange("p (n e) -> p n e", e=512), otb)

            for i in range(min(4, TT)):
                stage_a(i)
            drain_transposes()
            if stages == 'A':
                dbg = op.tile([128, 512], F32, tag="dbg", name="dbg")
                nc.vector.tensor_copy(dbg[:, 0:128], kT[0])
                nc.sync.dma_start(out_d.ap()[0:128, 0:512], dbg)
            else:
                # flat stream of rep*TQ groups: group g+1's strips are
                # prefetched/computed inside bc(g), ACROSS rep boundaries,
                # so the rep-slope has no per-rep pipeline refill
                NG = rep * TQ
                for g in range(NG):
                    j = g % TQ

                    def strips(gg):
                        if gg >= NG:
                            return []
                        return [(4 * gg + k) % TT for k in range(4)]

                    nxts = strips(g + 1)       # computed inside bc(g)
                    nxtl = strips(g + 2)       # loaded inside bc(g)
                    if deep:
                        stage_bc(j, nxts, nxtl)
                    else:
                        for i in nxts:
                            load_strip(i)
                        stage_bc(j, [], [])
                        for i in nxts:
                            stage_a(i)
                        drain_transposes()
    nc.compile()
    return nc


def make_core_inputs(x, cos, sin, wq, wk, wv, wproj):
    """Full inputs -> list of 8 per-core input dicts (host-side sharding)."""
    bf16 = mybir.dt.np(BF16)
    f8 = mybir.dt.np(FP8)
    xdt = f8 if FP8P else bf16
    pdt = f8 if FP8C else bf16
    wscale = W_SCALE if FP8P else 1.0
    pscale = W_SCALE if FP8C else 1.0
    x = np.asarray(x, dtype=np.float32)
    cos2 = np.ascontiguousarray(np.asarray(cos, np.float32).reshape(-1, D))
    sin2 = np.ascontiguousarray(np.asarray(sin, np.float32).reshape(-1, D))
    wq = np.asarray(wq, np.float32)
    wk = np.asarray(wk, np.float32)
    wv = np.asarray(wv, np.float32)
    wproj = np.asarray(wproj, np.float32)
    B = x.shape[0]
    tri = np.triu(np.ones((128, 128), np.float32))
    if PEMASK:
        mask = np.where(tri > 0, 0.0, -30000.0).astype(np.float32).astype(bf16)
    else:
        mask = tri.astype(bf16)
    ident = np.eye(128, dtype=np.float32).astype(bf16)
    in_maps = []
    xTs = [np.ascontiguousarray(x[b].T).astype(xdt) for b in range(B)]
    for b in range(B):
        for g in range(N_KV):
            wqT = np.ascontiguousarray(
                wq[512 * g:512 * g + 512].T * wscale).astype(xdt)
            wkvT = np.ascontiguousarray(
                np.concatenate([wk[128 * g:128 * g + 128],
                                wv[128 * g:128 * g + 128]],
                               axis=0).T * wscale).astype(xdt)
            wpT = np.ascontiguousarray(
                wproj[:, 512 * g:512 * g + 512].T * pscale).astype(pdt)
            in_maps.append({
                "xT": xTs[b], "wqT": wqT, "wkvT": wkvT, "wpT": wpT,
                "cosd": cos2, "sind": sin2, "maskd": mask, "identd": ident,
            })
    return in_maps


_NC_CACHE = {}


def kernel(x, cos, sin, wq, wk, wv, wproj):
    x = np.asarray(x, dtype=np.float32)
    B, T, C = x.shape
    key = (T, C)
    if key not in _NC_CACHE:
        _NC_CACHE[key] = build_bass(T=T, C=C)
    nc = _NC_CACHE[key]
    in_maps = make_core_inputs(x, cos, sin, wq, wk, wv, wproj)
    res = run_bass_kernel_spmd(nc, in_maps, core_ids=list(range(8)))
    out = np.zeros((B, T, C), dtype=np.float64)
    for b in range(B):
        for g in range(N_KV):
            out[b] += res.results[4 * b + g]["out"].astype(np.float64)
    return out.astype(np.float32)


# revision 49
# speedup vs baseline: 1.1149x; 1.1149x over previous
"""Trainium2 Bass kernel: causal GQA self-attention, RoPE + QK RMS-norm, bf16.

Sharding over 8 NeuronCores: core = 4*b + g (b in {0,1} batch, g in {0..3}
kv-group). Each core computes its 4 q heads + 1 kv head and the partial
c_proj output y_heads @ wproj[:, 512g:512g+512].T of shape [T, C]; the host
sums the 4 partials per batch (the "all-reduce after c_proj" at gather time).

Schedule (deep software pipeline): stage A of strip 4j+4+h is emitted inside
stage B(j) between heads h and h+1, so the serial per-strip chain
(PE proj -> DVE rope+rsqrt -> xbar transpose) hides under the PE-heavy
attention sweeps and every engine queue keeps flowing. Choices that matter:
 - rsqrt for the QK RMS-norm runs ENTIRELY on DVE (bit-trick seed + 2
   Newton steps on [128,5] tiles): the ACT engine then only ever runs
   Exp/Copy, which share one activation-table set -> zero ACT_TABLE_LOADs
   after the first.
 - causal mask is an additive -30000 matmul folded into the scores psum
   accumulation group on PE (no post-exp masking work on DVE/ACT).
 - q transposes are batched: one 3D-dst xbar DMA per strip covers all 4
   heads ([t,(h d)] -> [d,h,t]).
 - output stores are 4 batched [128,2048] DMAs per slice on the gpsimd
   (SWDGE) queue; cos/sin loads ride the same queue ahead of them; the SP
   (HWDGE) queue carries only x strips + transposes. In-order DMA queues
   head-of-line block on data dependencies, so queue assignment is part of
   the schedule.
 - fp8 (DoubleRow) paths exist but are OFF: every naive e4m3 insertion
   measures ~2.8e-2 rel err alone (gate 2e-2); error-feedback splits cost
   back the 2x.
"""

import math
from contextlib import ExitStack

import numpy as np

import concourse.bass as bass
import concourse.mybir as mybir
import concourse.tile as tile
from concourse import bacc
from concourse.bass import ts
from concourse.bass_utils import run_bass_kernel_spmd

F32 = mybir.dt.float32
I32 = mybir.dt.int32
BF16 = mybir.dt.bfloat16
FP8 = mybir.dt.float8e4
N_HEAD = 16
N_KV = 4
D = 128
RMS_EPS = float(np.finfo(np.float32).eps)
SCALE = 1.0 / math.sqrt(D)

FP8P = False         # x/wq/wkv in fp8, DoubleRow projections (see docstring)
FP8C = False         # wproj/ynj in fp8, DoubleRow c_proj
BFRO = True          # rope in bf16: cos/sin shipped bf16 from the host
W_SCALE = 64.0
OUT_DT = "bf16"      # halves output-store DMA; measured rel err impact
                     # 3.6e-3 -> 4.5e-3 in the numpy quantization model
PEMASK = True        # mask = additive -30000 folded into scores matmul group

ALU = mybir.AluOpType
AF = mybir.ActivationFunctionType


def build_bass(T=2048, C=2048, HQ=4, E=2048, rep=1,
               pscfg=None, pbufs=16, obufs=3,
               stages='ABC', wsplit=4, gdma=True,
               defnorm=True, fp8p=None, fp8c=None, outdt=None,
               nrsqrt=True, deep=True, bigot=True, bxp=True,
               pemask=True, csq='gpsimd', stq='gpsimd', nriter=2,
               bfro=None, otact=True, pfd=2):
    TT, CT, NE, TQ = T // 128, C // 128, E // 512, T // 512
    HD = HQ * 128
    if fp8p is None:
        fp8p = FP8P
    if fp8c is None:
        fp8c = FP8C
    if outdt is None:
        outdt = OUT_DT
    if bfro is None:
        bfro = BFRO
    dt = BF16
    xdt = FP8 if fp8p else BF16
    ydt = FP8 if fp8c else BF16
    odt = F32 if outdt == "f32" else BF16
    os_val = (1.0 / W_SCALE) if fp8p else 1.0
    ot_scale = (1.0 / W_SCALE) if fp8c else None
    if pscfg is None:
        pscfg = (3, 1, 1, 2, 1)  # ps_s, pq, pkv, ps_a, ps_d (8 banks)
    sb_, qb_, kb_, ab_, db_ = pscfg

    nc = bacc.Bacc("TRN2", target_bir_lowering=False)
    xT_d = nc.dram_tensor("xT", [C, T], xdt, kind="ExternalInput")
    wqT_d = nc.dram_tensor("wqT", [C, HD], xdt, kind="ExternalInput")
    wkvT_d = nc.dram_tensor("wkvT", [C, 256], xdt, kind="ExternalInput")
    wpT_d = nc.dram_tensor("wpT", [HD, E], ydt, kind="ExternalInput")
    csdt = BF16 if bfro else F32  # bf16 trig shipped from the host
    cos_d = nc.dram_tensor("cosd", [T, D], csdt, kind="ExternalInput")
    sin_d = nc.dram_tensor("sind", [T, D], csdt, kind="ExternalInput")
    mask_d = nc.dram_tensor("maskd", [128, 128], dt, kind="ExternalInput")
    id_d = nc.dram_tensor("identd", [128, 128], dt, kind="ExternalInput")
    out_d = nc.dram_tensor("out", [T, E], odt, kind="ExternalOutput")

    DR = mybir.MatmulPerfMode.DoubleRow
    assert pemask == PEMASK, "host mask content must match pemask"

    with tile.TileContext(nc) as tc, ExitStack() as ctx:
        P = lambda **kw: ctx.enter_context(tc.tile_pool(**kw))
        wp = P(name="w", bufs=1)            # persistent weights/constants
        xp = P(name="x", bufs=8)            # xT strips
        csp = P(name="cs", bufs=8)          # cos/sin tiles
        rp = P(name="rope", bufs=3)         # rope scratch
        qnp = P(name="qn", bufs=3)          # normalized q/k (pre-transpose)
        pp = P(name="p", bufs=pbufs)        # exp(P) tiles (all blocks alive)
        bp = P(name="bc", bufs=2)           # denominators / bcast
        yp = P(name="y", bufs=3)            # per-head unnormalized y^T
        op = P(name="o", bufs=obufs)        # output staging
        ps_s = P(name="ps_s", bufs=sb_, space="PSUM")   # scores/pb/cproj
        ps_q = P(name="ps_q", bufs=qb_, space="PSUM")   # q projection
        ps_k = P(name="ps_k", bufs=kb_, space="PSUM")   # kv projection
        ps_a = P(name="ps_a", bufs=ab_, space="PSUM")   # AV accumulators
        ps_d = P(name="ps_d", bufs=db_, space="PSUM")   # denominators

        engq = {"gpsimd": nc.gpsimd, "sp": nc.sync,
                "act": nc.scalar, "dve": nc.vector}
        cse = engq[csq] if gdma else nc.sync
        ste = engq[stq] if gdma else nc.sync

        xT_r = xT_d.ap().rearrange("(n p) t -> p n t", p=128)
        xs_t, cs_t = {}, {}

        def load_strip(i):
            xs = xp.tile([128, CT, 128], xdt, name="xs", tag="xs")
            nc.sync.dma_start(xs, xT_r[:, :, ts(i, 128)])
            cst = csp.tile([128, D], csdt, tag="cos", name="cst")
            cse.dma_start(cst, cos_d.ap()[ts(i, 128), :])
            snt = csp.tile([128, D], csdt, tag="sin", name="snt")
            cse.dma_start(snt, sin_d.ap()[ts(i, 128), :])
            xs_t[i] = xs
            cs_t[i] = (cst, snt)

        # ---- startup DMAs: first strip + first weight chunk, then the
        # rest; strips 4..7 prefetch right behind so group 1's stage A
        # (interleaved into bc(0)) never waits on the SP queue ----
        load_strip(0)
        wq_s = wp.tile([128, CT, HD], xdt)
        wkv_s = wp.tile([128, CT, 256], xdt)
        wq_r = wqT_d.ap().rearrange("(n p) m -> p n m", p=128)
        wkv_r = wkvT_d.ap().rearrange("(n p) m -> p n m", p=128)
        csz = CT // wsplit
        nc.sync.dma_start(wq_s[:, 0:csz], wq_r[:, 0:csz])
        nc.sync.dma_start(wkv_s[:, 0:csz], wkv_r[:, 0:csz])
        mask_s = wp.tile([128, 128], dt)
        nc.sync.dma_start(mask_s, mask_d.ap())
        ident = wp.tile([128, 128], dt)
        nc.sync.dma_start(ident, id_d.ap())
        for w in range(1, wsplit):
            cs0 = w * csz
            nc.sync.dma_start(wq_s[:, cs0:cs0 + csz], wq_r[:, cs0:cs0 + csz])
            nc.sync.dma_start(wkv_s[:, cs0:cs0 + csz], wkv_r[:, cs0:cs0 + csz])
        for i in range(1, min(4 * pfd, TT)):
            load_strip(i)
        wp_s = wp.tile([128, HQ, E], ydt)
        wp_r = wpT_d.ap().rearrange("(n p) m -> p n m", p=128)
        wp_loaded = [False]
        ones_c = wp.tile([128, 1], dt, name="ones_c", tag="ones_c")
        nc.vector.memset(ones_c, 1.0)
        ones_sq = wp.tile([128, 128], dt, name="ones_sq", tag="ones_sq")
        nc.vector.memset(ones_sq, os_val)
        eps_s = wp.tile([128, 1], F32)
        nc.vector.memset(eps_s, RMS_EPS)

        def bcast(ap, axis, n):
            a = list(ap.ap)
            a.insert(axis, [0, n])
            return bass.AP(tensor=ap.tensor, offset=ap.offset, ap=a)

        if True:  # tiles/closures shared by all reps (flat group stream)
            if bxp:
                qT = {j: wp.tile([128, HQ, 4, 128], dt, tag=f"qTj{j}",
                                 name=f"qTj{j}") for j in range(TQ)}
            else:
                qT = {}
                for h in range(HQ):
                    for j in range(TQ):
                        qT[(h, j)] = wp.tile([128, 4, 128], dt,
                                             tag=f"qT{h}_{j}",
                                             name=f"qT{h}_{j}")
            kT = [wp.tile([128, 128], dt, tag=f"kT{i}", name=f"kT{i}")
                  for i in range(TT)]
            vS = [wp.tile([128, 128], dt, tag=f"vS{i}", name=f"vS{i}")
                  for i in range(TT)]

            pend = []  # deferred transposes: (src_ap, dst)

            def drain_transposes():
                for src_ap, dst in pend:
                    nc.sync.dma_start(dst, src_ap, transpose=True)
                del pend[:]

            rdt = dt if bfro else F32

            def rope(src, nh, cst, snt, qn, qo):
                """src: [128, nh, 128] (bf16 sbuf if bfro, else psum f32);
                returns the rotated tile ro."""
                ro = rp.tile([128, nh, D], rdt, tag=f"ro{qo}", name="ro")
                nc.vector.tensor_mul(ro, src, bcast(cst[:, :], 1, nh))
                tmp = rp.tile([128, nh, 64], rdt, tag=f"tm{qo}", name="tmp")
                nc.vector.tensor_mul(tmp, src[:, :, 64:128],
                                     bcast(snt[:, 0:64], 1, nh))
                nc.vector.tensor_sub(ro[:, :, 0:64], ro[:, :, 0:64], tmp)
                tmp2 = rp.tile([128, nh, 64], rdt, tag=f"t2{qo}", name="tmp2")
                nc.vector.tensor_mul(tmp2, src[:, :, 0:64],
                                     bcast(snt[:, 64:128], 1, nh))
                nc.vector.tensor_add(ro[:, :, 64:128], ro[:, :, 64:128],
                                     tmp2)
                return ro

            MAGIC = 0x5F3759DF

            def nr_rsqrt(rr, v):
                """rr = 1/sqrt(v) elementwise on DVE only ([128, n] tiles).

                Bit-trick seed y0 via (M2 + ~i) >> 1 (= magic - (i>>1) up to
                1 ulp of the seed), then `nriter` Newton steps; no ACT
                involvement so the activation table stays on the Exp set.
                """
                n = v.shape[1]
                y = rp.tile([128, n], F32, tag="nr_y", name="nr_y")
                vb = v.bitcast(I32)
                yb = y.bitcast(I32)
                # walrus requires op0/op1 of one tensor_scalar to share a
                # class (bitwise vs arith), and there is no reversed
                # subtract: use magic - (i>>1) = ~(i>>1) + (magic+1)
                nc.vector.tensor_scalar(yb, vb, 1, 0xFFFFFFFF,
                                        ALU.logical_shift_right,
                                        ALU.bitwise_xor)
                nc.vector.tensor_scalar(yb, yb, MAGIC + 1, None, ALU.add)
                t = rp.tile([128, n], F32, tag="nr_t", name="nr_t")
                for _ in range(nriter):
                    nc.vector.tensor_mul(t, y, y)
                    nc.vector.tensor_mul(t, t, v)
                    nc.vector.tensor_scalar(t, t, -0.5, 1.5,
                                            ALU.mult, ALU.add)
                    nc.vector.tensor_mul(y, y, t)
                nc.vector.tensor_copy(rr, y)

            def stage_a(i):
                drain_transposes()
                xs = xs_t[i]
                cst, snt = cs_t[i]
                if not wp_loaded[0] and i >= 1:
                    # all HQ wproj chunks must be queued on the SP FIFO
                    # before bc(0)'s output stores (stage C(0) reads them)
                    hi = HQ if i >= 3 else i
                    for w in range(i - 1, hi):
                        nc.sync.dma_start(wp_s[:, w], wp_r[:, w])
                    if i >= 3:
                        wp_loaded[0] = True

                pq = ps_q.tile([128, HD], F32, tag="pq", name="pq")
                pkv = ps_k.tile([128, 256], F32, tag="pkv", name="pkv")
                if fp8p:
                    n2 = CT // 2
                    for c in range(n2):
                        sl = slice(2 * c, 2 * c + 2)
                        nc.tensor.matmul(pq, xs[:, sl], wq_s[:, sl],
                                         start=(c == 0), stop=(c == n2 - 1),
                                         perf_mode=DR)
                        nc.tensor.matmul(pkv, xs[:, sl], wkv_s[:, sl],
                                         start=(c == 0), stop=(c == n2 - 1),
                                         perf_mode=DR)
                else:
                    for c in range(CT):
                        nc.tensor.matmul(pq, xs[:, c], wq_s[:, c],
                                         start=(c == 0), stop=(c == CT - 1))
                        nc.tensor.matmul(pkv, xs[:, c], wkv_s[:, c],
                                         start=(c == 0), stop=(c == CT - 1))
                nc.scalar.copy(vS[i], pkv[:, 128:256])
                j, tsub = i // 4, i % 4

                if bfro:
                    # ACT copies psum->sbuf bf16 up front: frees the psum
                    # bank after ~0.6us and every rope op then runs in the
                    # DVE 2-byte fast mode on SBUF
                    qsrc = rp.tile([128, HQ, D], dt, tag="qsrc", name="qsrc")
                    nc.scalar.copy(
                        qsrc.rearrange("p a b -> p (a b)"), pq[:])
                    ksrc = rp.tile([128, 1, D], dt, tag="ksrc", name="ksrc")
                    nc.scalar.copy(
                        ksrc.rearrange("p a b -> p (a b)"), pkv[:, 0:128])
                    q_in = qsrc[:]
                    k_in = ksrc[:]
                else:
                    q_in = pq[:].rearrange("p (h d) -> p h d", d=D)
                    k_in = pkv[:, 0:128].rearrange("p (h d) -> p h d", d=D)

                ro_q = rope(q_in, HQ, cst, snt, None, 0)
                ro_k = rope(k_in, 1, cst, snt, None, 8)
                sq5 = rp.tile([128, HQ + 1], rdt, tag="sq5", name="sq5")
                scr = rp.tile([128, HQ, D], rdt, tag="scr", name="scr")
                nc.vector.tensor_mul(scr, ro_q, ro_q)
                scrk = rp.tile([128, 1, D], rdt, tag="scrk", name="scrk")
                nc.vector.tensor_mul(scrk, ro_k, ro_k)
                with nc.allow_low_precision(
                        "rms sumsq in bf16: 0.4% on the q/k scale, "
                        "measured 4.5e-3 -> 5.4e-3 end to end"):
                    nc.vector.reduce_sum(sq5[:, 0:HQ], scr,
                                         axis=mybir.AxisListType.X)
                    nc.vector.reduce_sum(sq5[:, HQ:HQ + 1], scrk,
                                         axis=mybir.AxisListType.X)
                rr5 = rp.tile([128, HQ + 1], F32, tag="rr5", name="rr5")
                if nrsqrt:
                    sq5f = rp.tile([128, HQ + 1], F32, tag="sq5f",
                                   name="sq5f")
                    nc.vector.tensor_scalar(sq5f, sq5, 1.0 / D, RMS_EPS,
                                            ALU.mult, ALU.add)
                    nr_rsqrt(rr5, sq5f)
                else:
                    nc.scalar.activation(rr5, sq5, AF.Abs_reciprocal_sqrt,
                                         bias=eps_s[:, :], scale=1.0 / D)

                qn = qnp.tile([128, HQ + 1, D], dt, tag="qn", name="qn")
                for h in range(HQ):
                    nc.vector.tensor_scalar_mul(qn[:, h], ro_q[:, h],
                                                rr5[:, h:h + 1])
                nc.vector.tensor_scalar_mul(qn[:, HQ], ro_k[:, 0],
                                            rr5[:, HQ:HQ + 1])
                if bxp:
                    pend.append((qn[:, 0:HQ].rearrange("p a b -> p (a b)"),
                                 qT[j][:, :, tsub]))
                    pend.append((qn[:, HQ], kT[i]))
                else:
                    for h in range(HQ):
                        pend.append((qn[:, h], qT[(h, j)][:, tsub]))
                    pend.append((qn[:, HQ], kT[i]))

            # ---- stage B + C per tq-slice ----
            def stage_bc(j, nxt, nxtl):
                nblk = 4 * j + 4
                ynj = yp.tile([128, HQ, 4, 128], ydt, tag="ynj", name="ynj")
                pend_norm = []  # deferred one head for slack

                def drain_norm():
                    for rdr_p, yv_p, h_p in pend_norm:
                        pb = ps_s.tile([128, 512], F32, tag="s", name="pb")
                        nc.tensor.matmul(pb, ones_sq[0:1, :], rdr_p)
                        nc.vector.tensor_mul(
                            ynj[:, h_p].rearrange("p a b -> p (a b)"),
                            yv_p, pb)
                    del pend_norm[:]

                for h in range(HQ):
                    if h < len(nxtl):
                        load_strip(nxtl[h])
                    pes = []
                    for i in range(nblk):
                        ai = max(0, i - 4 * j) * 128
                        diag = i >= 4 * j
                        psb = ps_s.tile([128, 512], F32, tag="s")
                        qmv = (qT[j][:, h, ai // 128:4] if bxp
                               else qT[(h, j)][:, ai // 128:4])
                        if pemask and diag:
                            nc.tensor.matmul(psb[:, ai:512], kT[i], qmv,
                                             start=True, stop=False)
                            nc.tensor.matmul(psb[:, ai:ai + 128], ident,
                                             mask_s, start=False, stop=True)
                        else:
                            nc.tensor.matmul(psb[:, ai:512], kT[i], qmv)
                        pe = pp.tile([128, 512], dt, tag="pe")
                        nc.scalar.activation(pe[:, ai:512], psb[:, ai:512],
                                             AF.Exp, scale=SCALE)
                        if diag and not pemask:
                            nc.vector.tensor_mul(pe[:, ai:ai + 128],
                                                 pe[:, ai:ai + 128], mask_s)
                        pes.append((pe, ai))
                    pdh = ps_d.tile([1, 512], F32)
                    for i, (pe, ai) in enumerate(pes):
                        nc.tensor.matmul(pdh[:, ai:512], ones_c,
                                         pe[:, ai:512],
                                         start=(i == 0), stop=(i == nblk - 1))
                    rd = bp.tile([1, 512], F32, tag="rd")
                    nc.vector.reciprocal(rd, pdh)
                    rdr = bp.tile([1, 512], dt, tag="rdr")
                    nc.vector.tensor_copy(rdr, rd)
                    if defnorm:
                        drain_norm()
                    pav = ps_a.tile([128, 512], F32)
                    for i, (pe, ai) in enumerate(pes):
                        nc.tensor.matmul(pav[:, ai:512], vS[i],
                                         pe[:, ai:512],
                                         start=(i == 0), stop=(i == nblk - 1))
                    yv = yp.tile([128, 512], dt, tag="yv", name="yv")
                    if h % 2 == 0:
                        nc.scalar.copy(yv, pav)
                    else:
                        nc.vector.tensor_copy(yv, pav)
                    pend_norm.append((rdr, yv, h))
                    if not defnorm or h == HQ - 1:
                        # last head: normalize BEFORE the interleaved
                        # stage_a so stage C's ynj read isn't queued on DVE
                        # behind a whole strip's rope chain
                        drain_norm()
                    if h < len(nxt):
                        stage_a(nxt[h])
                drain_norm()
                drain_transposes()
                if stages == 'AB':
                    dbg2 = op.tile([128, 512], F32, tag="dbg")
                    nc.vector.tensor_copy(
                        dbg2, ynj[:, 0].rearrange("p a b -> p (a b)"))
                    nc.sync.dma_start(out_d.ap()[ts(j, 128), 0:512], dbg2)
                    return
                for tsub in range(4):
                    otb = None
                    if bigot:
                        otb = op.tile([128, NE, 512], odt, tag="ot",
                                      name="otb")
                    for e in range(NE):
                        pc = ps_s.tile([128, 512], F32, tag="s", name="pc")
                        if fp8c:
                            for hp in range(HQ // 2):
                                nc.tensor.matmul(
                                    pc, ynj[:, 2 * hp:2 * hp + 2, tsub],
                                    wp_s[:, 2 * hp:2 * hp + 2, ts(e, 512)],
                                    start=(hp == 0), stop=(hp == HQ // 2 - 1),
                                    perf_mode=DR)
                        else:
                            for h in range(HQ):
                                nc.tensor.matmul(pc, ynj[:, h, tsub],
                                                 wp_s[:, h, ts(e, 512)],
                                                 start=(h == 0),
                                                 stop=(h == HQ - 1))
                        ot = otb[:, e] if bigot else op.tile(
                            [128, 512], odt, tag="ot", name="ot")
                        if otact or e % 2 == 0:
                            if ot_scale is None:
                                nc.scalar.copy(ot, pc)
                            else:
                                nc.scalar.mul(ot, pc, ot_scale)
                        else:
                            if ot_scale is None:
                                nc.vector.tensor_copy(ot, pc)
                            else:
                                nc.vector.tensor_scalar_mul(ot, pc, ot_scale)
                        if not bigot:
                            deng = (nc.sync if (e % 2 == 0 or not gdma)
                                    else nc.gpsimd)
                            deng.dma_start(
                                out_d.ap()[512 * j + 128 * tsub:
                                           512 * j + 128 * tsub + 128,
                                           ts(e, 512)], ot)
                    if bigot:
                        ste.dma_start(
                            out_d.ap()[512 * j + 128 * tsub:
                                       512 * j + 128 * tsub + 128, :]
                            .rearrange("p (n e) -> p n e", e=512), otb)

            for i in range(min(4, TT)):
                stage_a(i)
            drain_transposes()
            if stages == 'A':
                dbg = op.tile([128, 512], F32, tag="dbg", name="dbg")
                nc.vector.tensor_copy(dbg[:, 0:128], kT[0])
                nc.sync.dma_start(out_d.ap()[0:128, 0:512], dbg)
            else:
                # flat stream of rep*TQ groups: group g+1's strips are
                # prefetched/computed inside bc(g), ACROSS rep boundaries,
                # so the rep-slope has no per-rep pipeline refill
                NG = rep * TQ
                for g in range(NG):
                    j = g % TQ

                    def strips(gg):
                        if gg >= NG:
                            return []
                        return [(4 * gg + k) % TT for k in range(4)]

                    nxts = strips(g + 1)       # computed inside bc(g)
                    nxtl = strips(g + pfd)     # loaded inside bc(g)
                    if deep:
                        stage_bc(j, nxts, nxtl)
                    else:
                        for i in nxts:
                            load_strip(i)
                        stage_bc(j, [], [])
                        for i in nxts:
                            stage_a(i)
                        drain_transposes()
    nc.compile()
    return nc


def make_core_inputs(x, cos, sin, wq, wk, wv, wproj):
    """Full inputs -> list of 8 per-core input dicts (host-side sharding)."""
    bf16 = mybir.dt.np(BF16)
    f8 = mybir.dt.np(FP8)
    xdt = f8 if FP8P else bf16
    pdt = f8 if FP8C else bf16
    wscale = W_SCALE if FP8P else 1.0
    pscale = W_SCALE if FP8C else 1.0
    x = np.asarray(x, dtype=np.float32)
    csnp = mybir.dt.np(BF16) if BFRO else np.float32
    cos2 = np.ascontiguousarray(
        np.asarray(cos, np.float32).reshape(-1, D)).astype(csnp)
    sin2 = np.ascontiguousarray(
        np.asarray(sin, np.float32).reshape(-1, D)).astype(csnp)
    wq = np.asarray(wq, np.float32)
    wk = np.asarray(wk, np.float32)
    wv = np.asarray(wv, np.float32)
    wproj = np.asarray(wproj, np.float32)
    B = x.shape[0]
    tri = np.triu(np.ones((128, 128), np.float32))
    if PEMASK:
        mask = np.where(tri > 0, 0.0, -30000.0).astype(np.float32).astype(bf16)
    else:
        mask = tri.astype(bf16)
    ident = np.eye(128, dtype=np.float32).astype(bf16)
    in_maps = []
    xTs = [np.ascontiguousarray(x[b].T).astype(xdt) for b in range(B)]
    for b in range(B):
        for g in range(N_KV):
            wqT = np.ascontiguousarray(
                wq[512 * g:512 * g + 512].T * wscale).astype(xdt)
            wkvT = np.ascontiguousarray(
                np.concatenate([wk[128 * g:128 * g + 128],
                                wv[128 * g:128 * g + 128]],
                               axis=0).T * wscale).astype(xdt)
            wpT = np.ascontiguousarray(
                wproj[:, 512 * g:512 * g + 512].T * pscale).astype(pdt)
            in_maps.append({
                "xT": xTs[b], "wqT": wqT, "wkvT": wkvT, "wpT": wpT,
                "cosd": cos2, "sind": sin2, "maskd": mask, "identd": ident,
            })
    return in_maps


_NC_CACHE = {}


def kernel(x, cos, sin, wq, wk, wv, wproj):
    x = np.asarray(x, dtype=np.float32)
    B, T, C = x.shape
    key = (T, C)
    if key not in _NC_CACHE:
        _NC_CACHE[key] = build_bass(T=T, C=C)
    nc = _NC_CACHE[key]
    in_maps = make_core_inputs(x, cos, sin, wq, wk, wv, wproj)
    res = run_bass_kernel_spmd(nc, in_maps, core_ids=list(range(8)))
    out = np.zeros((B, T, C), dtype=np.float64)
    for b in range(B):
        for g in range(N_KV):
            out[b] += res.results[4 * b + g]["out"].astype(np.float64)
    return out.astype(np.float32)


# revision 51
# speedup vs baseline: 1.1428x; 1.0250x over previous
"""Trainium2 Bass kernel: causal GQA self-attention, RoPE + QK RMS-norm, bf16.

Sharding over 8 NeuronCores: core = 4*b + g (b in {0,1} batch, g in {0..3}
kv-group). Each core computes its 4 q heads + 1 kv head and the partial
c_proj output y_heads @ wproj[:, 512g:512g+512].T of shape [T, C]; the host
sums the 4 partials per batch (the "all-reduce after c_proj" at gather time).

Schedule (deep software pipeline): stage A of strip 4j+4+h is emitted inside
stage B(j) between heads h and h+1, so the serial per-strip chain
(PE proj -> DVE rope+rsqrt -> xbar transpose) hides under the PE-heavy
attention sweeps and every engine queue keeps flowing. Choices that matter:
 - rsqrt for the QK RMS-norm runs ENTIRELY on DVE (bit-trick seed +
   Newton steps on [128,5] tiles): the ACT engine then only ever runs
   Exp/Copy, which share one activation-table set -> zero ACT_TABLE_LOADs
   after the first.
 - causal mask is an additive -30000 matmul folded into the scores psum
   accumulation group on PE (no post-exp masking work on DVE/ACT).
 - q transposes are batched: one 3D-dst xbar DMA per strip covers all 4
   heads ([t,(h d)] -> [d,h,t]).
 - output stores are 4 batched [128,2048] DMAs per slice on the gpsimd
   (SWDGE) queue; cos/sin loads ride the same queue ahead of them; the SP
   (HWDGE) queue carries only x strips + transposes. In-order DMA queues
   head-of-line block on data dependencies, so queue assignment is part of
   the schedule.
 - fp8 (DoubleRow) paths exist but are OFF: every naive e4m3 insertion
   measures ~2.8e-2 rel err alone (gate 2e-2); error-feedback splits cost
   back the 2x.
"""

import math
from contextlib import ExitStack

import numpy as np

import concourse.bass as bass
import concourse.mybir as mybir
import concourse.tile as tile
from concourse import bacc
from concourse.bass import ts
from concourse.bass_utils import run_bass_kernel_spmd

F32 = mybir.dt.float32
I32 = mybir.dt.int32
BF16 = mybir.dt.bfloat16
FP8 = mybir.dt.float8e4
N_HEAD = 16
N_KV = 4
D = 128
RMS_EPS = float(np.finfo(np.float32).eps)
SCALE = 1.0 / math.sqrt(D)

FP8P = False         # x/wq/wkv in fp8, DoubleRow projections (see docstring)
FP8C = False         # wproj/ynj in fp8, DoubleRow c_proj
BFRO = True          # rope in bf16: cos/sin shipped bf16 from the host
W_SCALE = 64.0
OUT_DT = "bf16"      # halves output-store DMA; measured rel err impact
                     # 3.6e-3 -> 4.5e-3 in the numpy quantization model
PEMASK = True        # mask = additive -30000 folded into scores matmul group

ALU = mybir.AluOpType
AF = mybir.ActivationFunctionType


def build_bass(T=2048, C=2048, HQ=4, E=2048, rep=1,
               pscfg=None, pbufs=16, obufs=3,
               stages='ABC', wsplit=4, gdma=True,
               defnorm=True, fp8p=None, fp8c=None, outdt=None,
               nrsqrt=True, deep=True, bigot=True, bxp=True,
               pemask=True, csq='gpsimd', stq='gpsimd', nriter=1,
               bfro=None, otact=True, pfd=2):
    TT, CT, NE, TQ = T // 128, C // 128, E // 512, T // 512
    HD = HQ * 128
    if fp8p is None:
        fp8p = FP8P
    if fp8c is None:
        fp8c = FP8C
    if outdt is None:
        outdt = OUT_DT
    if bfro is None:
        bfro = BFRO
    dt = BF16
    xdt = FP8 if fp8p else BF16
    ydt = FP8 if fp8c else BF16
    odt = F32 if outdt == "f32" else BF16
    os_val = (1.0 / W_SCALE) if fp8p else 1.0
    ot_scale = (1.0 / W_SCALE) if fp8c else None
    if pscfg is None:
        pscfg = (3, 1, 1, 2, 1)  # ps_s, pq, pkv, ps_a, ps_d (8 banks)
    sb_, qb_, kb_, ab_, db_ = pscfg

    nc = bacc.Bacc("TRN2", target_bir_lowering=False)
    xT_d = nc.dram_tensor("xT", [C, T], xdt, kind="ExternalInput")
    wqT_d = nc.dram_tensor("wqT", [C, HD], xdt, kind="ExternalInput")
    wkvT_d = nc.dram_tensor("wkvT", [C, 256], xdt, kind="ExternalInput")
    wpT_d = nc.dram_tensor("wpT", [HD, E], ydt, kind="ExternalInput")
    csdt = BF16 if bfro else F32  # bf16 trig shipped from the host
    cos_d = nc.dram_tensor("cosd", [T, D], csdt, kind="ExternalInput")
    sin_d = nc.dram_tensor("sind", [T, D], csdt, kind="ExternalInput")
    mask_d = nc.dram_tensor("maskd", [128, 128], dt, kind="ExternalInput")
    id_d = nc.dram_tensor("identd", [128, 128], dt, kind="ExternalInput")
    out_d = nc.dram_tensor("out", [T, E], odt, kind="ExternalOutput")

    DR = mybir.MatmulPerfMode.DoubleRow
    assert pemask == PEMASK, "host mask content must match pemask"

    with tile.TileContext(nc) as tc, ExitStack() as ctx:
        P = lambda **kw: ctx.enter_context(tc.tile_pool(**kw))
        wp = P(name="w", bufs=1)            # persistent weights/constants
        xp = P(name="x", bufs=8)            # xT strips
        csp = P(name="cs", bufs=8)          # cos/sin tiles
        rp = P(name="rope", bufs=3)         # rope scratch
        qnp = P(name="qn", bufs=3)          # normalized q/k (pre-transpose)
        pp = P(name="p", bufs=pbufs)        # exp(P) tiles (all blocks alive)
        bp = P(name="bc", bufs=2)           # denominators / bcast
        yp = P(name="y", bufs=3)            # per-head unnormalized y^T
        op = P(name="o", bufs=obufs)        # output staging
        ps_s = P(name="ps_s", bufs=sb_, space="PSUM")   # scores/pb/cproj
        ps_q = P(name="ps_q", bufs=qb_, space="PSUM")   # q projection
        ps_k = P(name="ps_k", bufs=kb_, space="PSUM")   # kv projection
        ps_a = P(name="ps_a", bufs=ab_, space="PSUM")   # AV accumulators
        ps_d = P(name="ps_d", bufs=db_, space="PSUM")   # denominators

        engq = {"gpsimd": nc.gpsimd, "sp": nc.sync,
                "act": nc.scalar, "dve": nc.vector}
        cse = engq[csq] if gdma else nc.sync
        ste = engq[stq] if gdma else nc.sync

        xT_r = xT_d.ap().rearrange("(n p) t -> p n t", p=128)
        xs_t, cs_t = {}, {}

        def load_strip(i):
            xs = xp.tile([128, CT, 128], xdt, name="xs", tag="xs")
            nc.sync.dma_start(xs, xT_r[:, :, ts(i, 128)])
            cst = csp.tile([128, D], csdt, tag="cos", name="cst")
            cse.dma_start(cst, cos_d.ap()[ts(i, 128), :])
            snt = csp.tile([128, D], csdt, tag="sin", name="snt")
            cse.dma_start(snt, sin_d.ap()[ts(i, 128), :])
            xs_t[i] = xs
            cs_t[i] = (cst, snt)

        # ---- startup DMAs: first strip + first weight chunk, then the
        # rest; strips 4..7 prefetch right behind so group 1's stage A
        # (interleaved into bc(0)) never waits on the SP queue ----
        load_strip(0)
        wq_s = wp.tile([128, CT, HD], xdt)
        wkv_s = wp.tile([128, CT, 256], xdt)
        wq_r = wqT_d.ap().rearrange("(n p) m -> p n m", p=128)
        wkv_r = wkvT_d.ap().rearrange("(n p) m -> p n m", p=128)
        csz = CT // wsplit
        nc.sync.dma_start(wq_s[:, 0:csz], wq_r[:, 0:csz])
        nc.sync.dma_start(wkv_s[:, 0:csz], wkv_r[:, 0:csz])
        mask_s = wp.tile([128, 128], dt)
        nc.sync.dma_start(mask_s, mask_d.ap())
        ident = wp.tile([128, 128], dt)
        nc.sync.dma_start(ident, id_d.ap())
        for w in range(1, wsplit):
            cs0 = w * csz
            nc.sync.dma_start(wq_s[:, cs0:cs0 + csz], wq_r[:, cs0:cs0 + csz])
            nc.sync.dma_start(wkv_s[:, cs0:cs0 + csz], wkv_r[:, cs0:cs0 + csz])
        for i in range(1, min(4 * pfd, TT)):
            load_strip(i)
        wp_s = wp.tile([128, HQ, E], ydt)
        wp_r = wpT_d.ap().rearrange("(n p) m -> p n m", p=128)
        wp_loaded = [False]
        ones_c = wp.tile([128, 1], dt, name="ones_c", tag="ones_c")
        nc.vector.memset(ones_c, 1.0)
        ones_sq = wp.tile([128, 128], dt, name="ones_sq", tag="ones_sq")
        nc.vector.memset(ones_sq, os_val)
        eps_s = wp.tile([128, 1], F32)
        nc.vector.memset(eps_s, RMS_EPS)

        def bcast(ap, axis, n):
            a = list(ap.ap)
            a.insert(axis, [0, n])
            return bass.AP(tensor=ap.tensor, offset=ap.offset, ap=a)

        if True:  # tiles/closures shared by all reps (flat group stream)
            if bxp:
                qT = {j: wp.tile([128, HQ, 4, 128], dt, tag=f"qTj{j}",
                                 name=f"qTj{j}") for j in range(TQ)}
            else:
                qT = {}
                for h in range(HQ):
                    for j in range(TQ):
                        qT[(h, j)] = wp.tile([128, 4, 128], dt,
                                             tag=f"qT{h}_{j}",
                                             name=f"qT{h}_{j}")
            kT = [wp.tile([128, 128], dt, tag=f"kT{i}", name=f"kT{i}")
                  for i in range(TT)]
            vS = [wp.tile([128, 128], dt, tag=f"vS{i}", name=f"vS{i}")
                  for i in range(TT)]

            pend = []  # deferred transposes: (src_ap, dst)

            def drain_transposes():
                for src_ap, dst in pend:
                    nc.sync.dma_start(dst, src_ap, transpose=True)
                del pend[:]

            rdt = dt if bfro else F32

            def rope(src, nh, cst, snt, qn, qo):
                """src: [128, nh, 128] (bf16 sbuf if bfro, else psum f32);
                returns the rotated tile ro."""
                ro = rp.tile([128, nh, D], rdt, tag=f"ro{qo}", name="ro")
                nc.vector.tensor_mul(ro, src, bcast(cst[:, :], 1, nh))
                tmp = rp.tile([128, nh, 64], rdt, tag=f"tm{qo}", name="tmp")
                nc.vector.tensor_mul(tmp, src[:, :, 64:128],
                                     bcast(snt[:, 0:64], 1, nh))
                nc.vector.tensor_sub(ro[:, :, 0:64], ro[:, :, 0:64], tmp)
                tmp2 = rp.tile([128, nh, 64], rdt, tag=f"t2{qo}", name="tmp2")
                nc.vector.tensor_mul(tmp2, src[:, :, 0:64],
                                     bcast(snt[:, 64:128], 1, nh))
                nc.vector.tensor_add(ro[:, :, 64:128], ro[:, :, 64:128],
                                     tmp2)
                return ro

            MAGIC = 0x5F3759DF

            def nr_rsqrt(rr, v):
                """rr = 1/sqrt(v) elementwise on DVE only ([128, n] tiles).

                Bit-trick seed y0 via (M2 + ~i) >> 1 (= magic - (i>>1) up to
                1 ulp of the seed), then `nriter` Newton steps; no ACT
                involvement so the activation table stays on the Exp set.
                """
                n = v.shape[1]
                y = rp.tile([128, n], F32, tag="nr_y", name="nr_y")
                vb = v.bitcast(I32)
                yb = y.bitcast(I32)
                # walrus requires op0/op1 of one tensor_scalar to share a
                # class (bitwise vs arith), and there is no reversed
                # subtract: use magic - (i>>1) = ~(i>>1) + (magic+1)
                nc.vector.tensor_scalar(yb, vb, 1, 0xFFFFFFFF,
                                        ALU.logical_shift_right,
                                        ALU.bitwise_xor)
                nc.vector.tensor_scalar(yb, yb, MAGIC + 1, None, ALU.add)
                t = rp.tile([128, n], F32, tag="nr_t", name="nr_t")
                for _ in range(nriter):
                    nc.vector.tensor_mul(t, y, y)
                    nc.vector.tensor_mul(t, t, v)
                    nc.vector.tensor_scalar(t, t, -0.5, 1.5,
                                            ALU.mult, ALU.add)
                    nc.vector.tensor_mul(y, y, t)
                nc.vector.tensor_copy(rr, y)

            def stage_a(i):
                drain_transposes()
                xs = xs_t[i]
                cst, snt = cs_t[i]
                if not wp_loaded[0] and i >= 1:
                    # all HQ wproj chunks must be queued on the SP FIFO
                    # before bc(0)'s output stores (stage C(0) reads them)
                    hi = HQ if i >= 3 else i
                    for w in range(i - 1, hi):
                        nc.sync.dma_start(wp_s[:, w], wp_r[:, w])
                    if i >= 3:
                        wp_loaded[0] = True

                pq = ps_q.tile([128, HD], F32, tag="pq", name="pq")
                pkv = ps_k.tile([128, 256], F32, tag="pkv", name="pkv")
                if fp8p:
                    n2 = CT // 2
                    for c in range(n2):
                        sl = slice(2 * c, 2 * c + 2)
                        nc.tensor.matmul(pq, xs[:, sl], wq_s[:, sl],
                                         start=(c == 0), stop=(c == n2 - 1),
                                         perf_mode=DR)
                        nc.tensor.matmul(pkv, xs[:, sl], wkv_s[:, sl],
                                         start=(c == 0), stop=(c == n2 - 1),
                                         perf_mode=DR)
                else:
                    for c in range(CT):
                        nc.tensor.matmul(pq, xs[:, c], wq_s[:, c],
                                         start=(c == 0), stop=(c == CT - 1))
                        nc.tensor.matmul(pkv, xs[:, c], wkv_s[:, c],
                                         start=(c == 0), stop=(c == CT - 1))
                nc.scalar.copy(vS[i], pkv[:, 128:256])
                j, tsub = i // 4, i % 4

                if bfro:
                    # ACT copies psum->sbuf bf16 up front: frees the psum
                    # bank after ~0.6us and every rope op then runs in the
                    # DVE 2-byte fast mode on SBUF
                    qsrc = rp.tile([128, HQ, D], dt, tag="qsrc", name="qsrc")
                    nc.scalar.copy(
                        qsrc.rearrange("p a b -> p (a b)"), pq[:])
                    ksrc = rp.tile([128, 1, D], dt, tag="ksrc", name="ksrc")
                    nc.scalar.copy(
                        ksrc.rearrange("p a b -> p (a b)"), pkv[:, 0:128])
                    q_in = qsrc[:]
                    k_in = ksrc[:]
                else:
                    q_in = pq[:].rearrange("p (h d) -> p h d", d=D)
                    k_in = pkv[:, 0:128].rearrange("p (h d) -> p h d", d=D)

                ro_q = rope(q_in, HQ, cst, snt, None, 0)
                ro_k = rope(k_in, 1, cst, snt, None, 8)
                sq5 = rp.tile([128, HQ + 1], rdt, tag="sq5", name="sq5")
                scr = rp.tile([128, HQ, D], rdt, tag="scr", name="scr")
                nc.vector.tensor_mul(scr, ro_q, ro_q)
                scrk = rp.tile([128, 1, D], rdt, tag="scrk", name="scrk")
                nc.vector.tensor_mul(scrk, ro_k, ro_k)
                with nc.allow_low_precision(
                        "rms sumsq in bf16: 0.4% on the q/k scale, "
                        "measured 4.5e-3 -> 5.4e-3 end to end"):
                    nc.vector.reduce_sum(sq5[:, 0:HQ], scr,
                                         axis=mybir.AxisListType.X)
                    nc.vector.reduce_sum(sq5[:, HQ:HQ + 1], scrk,
                                         axis=mybir.AxisListType.X)
                rr5 = rp.tile([128, HQ + 1], F32, tag="rr5", name="rr5")
                if nrsqrt:
                    sq5f = rp.tile([128, HQ + 1], F32, tag="sq5f",
                                   name="sq5f")
                    nc.vector.tensor_scalar(sq5f, sq5, 1.0 / D, RMS_EPS,
                                            ALU.mult, ALU.add)
                    nr_rsqrt(rr5, sq5f)
                else:
                    nc.scalar.activation(rr5, sq5, AF.Abs_reciprocal_sqrt,
                                         bias=eps_s[:, :], scale=1.0 / D)

                qn = qnp.tile([128, HQ + 1, D], dt, tag="qn", name="qn")
                for h in range(HQ):
                    nc.vector.tensor_scalar_mul(qn[:, h], ro_q[:, h],
                                                rr5[:, h:h + 1])
                nc.vector.tensor_scalar_mul(qn[:, HQ], ro_k[:, 0],
                                            rr5[:, HQ:HQ + 1])
                if bxp:
                    pend.append((qn[:, 0:HQ].rearrange("p a b -> p (a b)"),
                                 qT[j][:, :, tsub]))
                    pend.append((qn[:, HQ], kT[i]))
                else:
                    for h in range(HQ):
                        pend.append((qn[:, h], qT[(h, j)][:, tsub]))
                    pend.append((qn[:, HQ], kT[i]))

            # ---- stage B + C per tq-slice ----
            def stage_bc(j, nxt, nxtl):
                nblk = 4 * j + 4
                ynj = yp.tile([128, HQ, 4, 128], ydt, tag="ynj", name="ynj")
                pend_norm = []  # deferred one head for slack

                def drain_norm():
                    for rdr_p, yv_p, h_p in pend_norm:
                        pb = ps_s.tile([128, 512], F32, tag="s", name="pb")
                        nc.tensor.matmul(pb, ones_sq[0:1, :], rdr_p)
                        nc.vector.tensor_mul(
                            ynj[:, h_p].rearrange("p a b -> p (a b)"),
                            yv_p, pb)
                    del pend_norm[:]

                for h in range(HQ):
                    if h < len(nxtl):
                        load_strip(nxtl[h])
                    pes = []
                    for i in range(nblk):
                        ai = max(0, i - 4 * j) * 128
                        diag = i >= 4 * j
                        psb = ps_s.tile([128, 512], F32, tag="s")
                        qmv = (qT[j][:, h, ai // 128:4] if bxp
                               else qT[(h, j)][:, ai // 128:4])
                        if pemask and diag:
                            nc.tensor.matmul(psb[:, ai:512], kT[i], qmv,
                                             start=True, stop=False)
                            nc.tensor.matmul(psb[:, ai:ai + 128], ident,
                                             mask_s, start=False, stop=True)
                        else:
                            nc.tensor.matmul(psb[:, ai:512], kT[i], qmv)
                        pe = pp.tile([128, 512], dt, tag="pe")
                        nc.scalar.activation(pe[:, ai:512], psb[:, ai:512],
                                             AF.Exp, scale=SCALE)
                        if diag and not pemask:
                            nc.vector.tensor_mul(pe[:, ai:ai + 128],
                                                 pe[:, ai:ai + 128], mask_s)
                        pes.append((pe, ai))
                    pdh = ps_d.tile([1, 512], F32)
                    for i, (pe, ai) in enumerate(pes):
                        nc.tensor.matmul(pdh[:, ai:512], ones_c,
                                         pe[:, ai:512],
                                         start=(i == 0), stop=(i == nblk - 1))
                    rd = bp.tile([1, 512], F32, tag="rd")
                    nc.vector.reciprocal(rd, pdh)
                    rdr = bp.tile([1, 512], dt, tag="rdr")
                    nc.vector.tensor_copy(rdr, rd)
                    if defnorm:
                        drain_norm()
                    pav = ps_a.tile([128, 512], F32)
                    for i, (pe, ai) in enumerate(pes):
                        nc.tensor.matmul(pav[:, ai:512], vS[i],
                                         pe[:, ai:512],
                                         start=(i == 0), stop=(i == nblk - 1))
                    yv = yp.tile([128, 512], dt, tag="yv", name="yv")
                    if h % 2 == 0:
                        nc.scalar.copy(yv, pav)
                    else:
                        nc.vector.tensor_copy(yv, pav)
                    pend_norm.append((rdr, yv, h))
                    if not defnorm or h == HQ - 1:
                        # last head: normalize BEFORE the interleaved
                        # stage_a so stage C's ynj read isn't queued on DVE
                        # behind a whole strip's rope chain
                        drain_norm()
                    if h < len(nxt):
                        stage_a(nxt[h])
                drain_norm()
                drain_transposes()
                if stages == 'AB':
                    dbg2 = op.tile([128, 512], F32, tag="dbg")
                    nc.vector.tensor_copy(
                        dbg2, ynj[:, 0].rearrange("p a b -> p (a b)"))
                    nc.sync.dma_start(out_d.ap()[ts(j, 128), 0:512], dbg2)
                    return
                for tsub in range(4):
                    otb = None
                    if bigot:
                        otb = op.tile([128, NE, 512], odt, tag="ot",
                                      name="otb")
                    for e in range(NE):
                        pc = ps_s.tile([128, 512], F32, tag="s", name="pc")
                        if fp8c:
                            for hp in range(HQ // 2):
                                nc.tensor.matmul(
                                    pc, ynj[:, 2 * hp:2 * hp + 2, tsub],
                                    wp_s[:, 2 * hp:2 * hp + 2, ts(e, 512)],
                                    start=(hp == 0), stop=(hp == HQ // 2 - 1),
                                    perf_mode=DR)
                        else:
                            for h in range(HQ):
                                nc.tensor.matmul(pc, ynj[:, h, tsub],
                                                 wp_s[:, h, ts(e, 512)],
                                                 start=(h == 0),
                                                 stop=(h == HQ - 1))
                        ot = otb[:, e] if bigot else op.tile(
                            [128, 512], odt, tag="ot", name="ot")
                        if otact or e % 2 == 0:
                            if ot_scale is None:
                                nc.scalar.copy(ot, pc)
                            else:
                                nc.scalar.mul(ot, pc, ot_scale)
                        else:
                            if ot_scale is None:
                                nc.vector.tensor_copy(ot, pc)
                            else:
                                nc.vector.tensor_scalar_mul(ot, pc, ot_scale)
                        if not bigot:
                            deng = (nc.sync if (e % 2 == 0 or not gdma)
                                    else nc.gpsimd)
                            deng.dma_start(
                                out_d.ap()[512 * j + 128 * tsub:
                                           512 * j + 128 * tsub + 128,
                                           ts(e, 512)], ot)
                    if bigot:
                        ste.dma_start(
                            out_d.ap()[512 * j + 128 * tsub:
                                       512 * j + 128 * tsub + 128, :]
                            .rearrange("p (n e) -> p n e", e=512), otb)

            for i in range(min(4, TT)):
                stage_a(i)
            drain_transposes()
            if stages == 'A':
                dbg = op.tile([128, 512], F32, tag="dbg", name="dbg")
                nc.vector.tensor_copy(dbg[:, 0:128], kT[0])
                nc.sync.dma_start(out_d.ap()[0:128, 0:512], dbg)
            else:
                # flat stream of rep*TQ groups: group g+1's strips are
                # prefetched/computed inside bc(g), ACROSS rep boundaries,
                # so the rep-slope has no per-rep pipeline refill
                NG = rep * TQ
                for g in range(NG):
                    j = g % TQ

                    def strips(gg):
                        if gg >= NG:
                            return []
                        return [(4 * gg + k) % TT for k in range(4)]

                    nxts = strips(g + 1)       # computed inside bc(g)
                    nxtl = strips(g + pfd)     # loaded inside bc(g)
                    if deep:
                        stage_bc(j, nxts, nxtl)
                    else:
                        for i in nxts:
                            load_strip(i)
                        stage_bc(j, [], [])
                        for i in nxts:
                            stage_a(i)
                        drain_transposes()
    nc.compile()
    return nc


def make_core_inputs(x, cos, sin, wq, wk, wv, wproj):
    """Full inputs -> list of 8 per-core input dicts (host-side sharding)."""
    bf16 = mybir.dt.np(BF16)
    f8 = mybir.dt.np(FP8)
    xdt = f8 if FP8P else bf16
    pdt = f8 if FP8C else bf16
    wscale = W_SCALE if FP8P else 1.0
    pscale = W_SCALE if FP8C else 1.0
    x = np.asarray(x, dtype=np.float32)
    csnp = mybir.dt.np(BF16) if BFRO else np.float32
    cos2 = np.ascontiguousarray(
        np.asarray(cos, np.float32).reshape(-1, D)).astype(csnp)
    sin2 = np.ascontiguousarray(
        np.asarray(sin, np.float32).reshape(-1, D)).astype(csnp)
    wq = np.asarray(wq, np.float32)
    wk = np.asarray(wk, np.float32)
    wv = np.asarray(wv, np.float32)
    wproj = np.asarray(wproj, np.float32)
    B = x.shape[0]
    tri = np.triu(np.ones((128, 128), np.float32))
    if PEMASK:
        mask = np.where(tri > 0, 0.0, -30000.0).astype(np.float32).astype(bf16)
    else:
        mask = tri.astype(bf16)
    ident = np.eye(128, dtype=np.float32).astype(bf16)
    in_maps = []
    xTs = [np.ascontiguousarray(x[b].T).astype(xdt) for b in range(B)]
    for b in range(B):
        for g in range(N_KV):
            wqT = np.ascontiguousarray(
                wq[512 * g:512 * g + 512].T * wscale).astype(xdt)
            wkvT = np.ascontiguousarray(
                np.concatenate([wk[128 * g:128 * g + 128],
                                wv[128 * g:128 * g + 128]],
                               axis=0).T * wscale).astype(xdt)
            wpT = np.ascontiguousarray(
                wproj[:, 512 * g:512 * g + 512].T * pscale).astype(pdt)
            in_maps.append({
                "xT": xTs[b], "wqT": wqT, "wkvT": wkvT, "wpT": wpT,
                "cosd": cos2, "sind": sin2, "maskd": mask, "identd": ident,
            })
    return in_maps


_NC_CACHE = {}


def kernel(x, cos, sin, wq, wk, wv, wproj):
    x = np.asarray(x, dtype=np.float32)
    B, T, C = x.shape
    key = (T, C)
    if key not in _NC_CACHE:
        _NC_CACHE[key] = build_bass(T=T, C=C)
    nc = _NC_CACHE[key]
    in_maps = make_core_inputs(x, cos, sin, wq, wk, wv, wproj)
    res = run_bass_kernel_spmd(nc, in_maps, core_ids=list(range(8)))
    out = np.zeros((B, T, C), dtype=np.float64)
    for b in range(B):
        for g in range(N_KV):
            out[b] += res.results[4 * b + g]["out"].astype(np.float64)
    return out.astype(np.float32)
